# revision 6
# baseline (speedup 1.0000x reference)
"""Trainium2 Bass kernel for a single attention head (nn_AttentionHead).

Problem: B=16, S=2048, W=768, H=64.
  Q = input @ Wq + bq ; K = input @ Wk + bk ; V = input @ Wv + bv
  scores = Q K^T / sqrt(H), key-padding mask, softmax, out = attn @ V.

Sharding: data-parallel over batch across 8 cores (2 samples per core).

Design (per core). Two cost facts drive it: TensorE matmul time
depends only on moving columns (contraction depth is free), and ScalarE
exp costs ~1.07 us per [128, 1024] tile. Both scale with the number of
KEY tiles, and masked keys (about half) contribute nothing - so the
host compacts each sample's keys to the valid subset (padded to whole
128-key tiles; pad keys get a -100 exp bias so they are exactly zero,
making compaction bit-equivalent).

  1. Host packs X^T bf16 tile-major [B, T, P, NW, 128] (full input for
     the Q pass; compacted keys for the K/V pass), stationaries
     Wq / [Wk|Wv], biases, and the exp bias table (layout prep only).
  2. Q projection (bf16, moving X^T) -> Q^T [64, S]; K/V projection
     (bf16, packed stationary, moving X_kv^T) -> kv [K^T rows 0:64 |
     V^T rows 64:128] over SKV compacted keys. DVE bias-add evacuations.
  3. Scores transposed S^T[k, q] = K^T.T Q^T, plain bf16 matmuls with
     64-deep contraction (cost is moving columns, so depth 64 is free).
  4. exp on ScalarE out of PSUM, scale=1/8 (absorbs 1/sqrt(H); weights
     stay unscaled), bias = -2 margin or -102 for pad keys; the margin
     cancels in the final divide.
  5. V' = [V | ones] rebuilt natural per key tile by TensorE transposes
     of kv rows 64:128 (identity corner at base partition 64). Attention
     runs in two query halves of 1024 so exp uses wide (1024-col)
     instructions while PSUM fits exactly: 2 score slots [128, 1024] +
     O'^T accumulator [65, 1024] + the projection ring = 8 banks. Row 64
     of O'^T is the softmax denominator; each half is evacuated as soon
     as it completes, overlapping the next half.
  6. Sample 1's entire prologue is deadline-scheduled into sample 0's
     attention iterations so TensorE/DVE/DMA work overlaps the exp
     stream without ever being emitted ahead of its producers.
  7. Host epilogue: O = O'[:64] / O'[64], transpose to [B, S, H].

Scheduling facts (measured): every dma_start pays ~0.6 us on the
issuing sequencer plus ~0.6 us on the SHARED HWDGE block, so input
loads are merged into a few large DMAs. DMA descriptors below 512
contiguous bytes run at half bus speed, and descriptor count drives
queue-feed latency - hence the tile-major DRAM layout, which gives
1.5 KB runs (one per partition per tile) instead of 256 B ones.
Weight/bias tensors load first, from the scalar queue's HWDGE (the
GPSIMD SWDGE path would land them behind the whole input stream).
The ACT exp table load (1.28 us) is hoisted by Bacc to the head of the
scalar queue (the dummy exp guarantees an early anchor). TensorE runs
at half clock for its first ~3 us of busy time (p-state ramp), so the
first projections start as early as the DMA plan allows. PE dispatch
is in-order with a 4-deep wait queue: a stage is only emitted once its
input DMA is expected to have landed, or it parks and stalls the pipe.
"""

import bisect
import functools

import ml_dtypes
import numpy as np

import concourse.bass as bass
import concourse.bacc as bacc
import concourse.mybir as mybir
import concourse.tile as tile
from concourse.bass_utils import run_bass_kernel_spmd
from concourse.masks import make_identity

F32 = mybir.dt.float32
BF16 = mybir.dt.bfloat16
AF = mybir.ActivationFunctionType
ALU = mybir.AluOpType

P = 128
B_PER_CORE = 2
S = 2048
W = 768
H = 64
NW = W // P      # 6 contraction chunks for the projections
NKT = S // P     # 16 key tiles uncompacted
NQC = S // 512   # 4 query chunks of 512
N_CORES = 8
PAD_BIAS = -100.0   # exp bias for pad keys (exp -> 0 exactly in bf16)
EXP_MARGIN = -2.0   # global exp bias margin (cancels in the divide)
QSCALE = 0.125      # 1/sqrt(H), applied as the exp scale

NP_BF16 = ml_dtypes.bfloat16


def _groups(nt, edges):
    """Tile-index groups [t0, t1) from edge list, clipped to nt tiles."""
    es = sorted(set([0] + [e for e in edges if 0 < e < nt] + [nt]))
    return list(zip(es[:-1], es[1:]))


def _emit_q_proj(nc, pools, b, qc):
    wq, bq, xt, qt, pps = (
        pools["wq"], pools["bq"], pools["xt"][b], pools["qt"][b], pools["pps"],
    )
    ps = pps.tile([P, 512], F32, tag="pps", name=f"pq_{b}_{qc}")
    for wc in range(NW):
        nc.tensor.matmul(
            ps[0:H, :],
            wq[:, wc, :],
            xt[:, 4 * qc : 4 * qc + 4, wc, :],
            start=(wc == 0),
            stop=(wc == NW - 1),
        )
    nc.vector.tensor_scalar(
        qt[:, qc * 512 : (qc + 1) * 512], ps[0:H, :], bq, None, ALU.add
    )


def _emit_kv_proj(nc, pools, b, t0, t1):
    """Project key tiles [t0, t1) (at most 4: moving free <= 512)."""
    wkv, bkv, xkv, kv, pps = (
        pools["wkv"], pools["bkv"], pools["xkv"][b], pools["kv"][b], pools["pps"],
    )
    n = (t1 - t0) * P
    ps = pps.tile([P, 512], F32, tag="pps", name=f"pkv_{b}_{t0}")
    for wc in range(NW):
        nc.tensor.matmul(
            ps[:, 0:n],
            wkv[:, wc, :],
            xkv[:, t0:t1, wc, :],
            start=(wc == 0),
            stop=(wc == NW - 1),
        )
    nc.vector.tensor_scalar(kv[:, t0 * P : t1 * P], ps[:, 0:n], bkv, None, ALU.add)


def _emit_vtrans(nc, pools, b, j, nkt_kv):
    """Transpose kv rows 64:128 (V^T) for key-tile pair (2j, 2j+1) into
    natural bf16 V' tiles; the last pair may hold a single tile."""
    kv, vp, ident, pps = (
        pools["kv"][b], pools["vp"][b], pools["ident"], pools["pps"],
    )
    n = min(2, nkt_kv - 2 * j)
    pst = pps.tile([P, P], BF16, tag="pps", name=f"pvt_{b}_{j}")
    for i in range(n):
        kt = 2 * j + i
        nc.tensor.transpose(
            pst[:, i * H : (i + 1) * H],
            kv[H:P, kt * P : (kt + 1) * P],
            ident[H:P, H:P],
        )
    nc.vector.tensor_copy(
        vp[:, 2 * j : 2 * j + n, 0:H],
        pst[:, 0 : n * H].rearrange("p (i h) -> p i h", h=H),
    )


def _emit_attention(nc, pools, b, out_e, nkt_kv, interleave=()):
    """Score -> exp -> PV loops for sample b, split into two query halves
    of 1024 (PSUM: two 1024-wide score slots + one [65, 1024] output
    accumulator + the projection ring = exactly 8 banks). interleave is a
    flat list over the 2*nkt_kv iterations; interleave[it] thunks are
    emitted at the top of that iteration (the other sample's prologue,
    filling TensorE under the exp stream)."""
    qt, kv, vp, ebias = (
        pools["qt"][b], pools["kv"][b], pools["vp"][b], pools["ebias"][b],
    )
    sps_p, ptp, pso_p, oup = pools["sps"], pools["ptp"], pools["pso"], pools["oup"]

    HQ = S // 2
    for half in range(2):
        pso = pso_p.tile([H + 1, HQ], F32, tag="pso", name=f"pso{b}_{half}")
        for kt in range(nkt_kv):
            it = half * nkt_kv + kt
            for thunk in (interleave[it] if it < len(interleave) else ()):
                thunk()
            pt = ptp.tile([P, HQ], BF16, tag="pt", name=f"pt_{b}_{it}")
            sps = sps_p.tile([P, HQ], F32, tag="sps", name=f"ss_{b}_{it}")
            for qi in range(2):
                nc.tensor.matmul(
                    sps[:, qi * 512 : (qi + 1) * 512],
                    kv[0:H, kt * P : (kt + 1) * P],
                    qt[:, half * HQ + qi * 512 : half * HQ + (qi + 1) * 512],
                    start=True,
                    stop=True,
                )
            nc.scalar.activation(
                pt, sps, AF.Exp, bias=ebias[:, kt : kt + 1], scale=QSCALE
            )
            for qi in range(2):
                nc.tensor.matmul(
                    pso[:, qi * 512 : (qi + 1) * 512],
                    vp[:, kt, :],
                    pt[:, qi * 512 : (qi + 1) * 512],
                    start=(kt == 0),
                    stop=(kt == nkt_kv - 1),
                )
        # evacuate this half right away (overlaps the next half / sample);
        # output DMA on the sync queue (HWDGE) - gpsimd SWDGE costs ~1 us
        # of Pool engine time per DMA and would stretch the tail.
        ou = oup.tile([H + 1, HQ], F32, tag="ou", name=f"ou{b}_{half}")
        for qi in range(2):
            sl = slice(qi * 512, (qi + 1) * 512)
            osl = slice(half * HQ + qi * 512, half * HQ + (qi + 1) * 512)
            nc.vector.tensor_copy(ou[:, sl], pso[:, sl])
            nc.sync.dma_start(out=out_e[b, :, osl], in_=ou[:, sl])


def _build(nc, tc, nkt_kv, xt_e, xkv_e, eb_e, wq_e, wkv_e, bq_e, bkv_e, out_e):
    skv = nkt_kv * P
    nit = 2 * nkt_kv
    with (
        tc.tile_pool(name="const", bufs=1) as cpool,
        tc.tile_pool(name="xtp", bufs=2) as xtp,
        tc.tile_pool(name="xkvp", bufs=2) as xkvp,
        tc.tile_pool(name="qtp", bufs=2) as qtp,
        tc.tile_pool(name="kvp", bufs=2) as kvp,
        tc.tile_pool(name="vpp", bufs=2) as vpp,
        tc.tile_pool(name="ptp", bufs=2) as ptp,
        tc.tile_pool(name="oup", bufs=2) as oup,
        tc.tile_pool(name="ebp", bufs=2) as ebp,
        tc.tile_pool(name="sps", bufs=2, space="PSUM") as sps_p,  # 2x[128,1024]
        tc.tile_pool(name="pps", bufs=2, space="PSUM") as pps,
        tc.tile_pool(name="psop", bufs=1, space="PSUM") as pso_p,
    ):
        prime = cpool.tile([1, 1], F32, name="prime", tag="prime")
        ident = cpool.tile([P, P], BF16, name="ident", tag="ident")
        wq = cpool.tile([P, NW, H], BF16, name="wq", tag="wq")
        wkv = cpool.tile([P, NW, P], BF16, name="wkv", tag="wkv")
        bq = cpool.tile([H, 1], F32, name="bq", tag="bq")
        bkv = cpool.tile([P, 1], F32, name="bkv", tag="bkv")

        # Pool queue: compute-only prep (no SWDGE DMAs -> cheap end drain)
        nc.gpsimd.memset(prime, 0.0)
        make_identity(nc, ident)

        pools = {
            "ident": ident, "wq": wq, "wkv": wkv, "bq": bq, "bkv": bkv,
            "sps": sps_p, "pps": pps, "pso": pso_p, "ptp": ptp, "oup": oup,
            "xt": [], "xkv": [], "qt": [], "kv": [], "vp": [], "ebias": [],
        }
        for b in range(B_PER_CORE):
            pools["ebias"].append(ebp.tile([P, nkt_kv], F32, tag="eb", name=f"eb{b}"))
            pools["xt"].append(
                xtp.tile([P, NKT, NW, P], BF16, tag="xt", name=f"xt{b}")
            )
            pools["xkv"].append(
                xkvp.tile([P, nkt_kv, NW, P], BF16, tag="xkv", name=f"xkv{b}")
            )
            pools["qt"].append(qtp.tile([H, S], BF16, tag="qt", name=f"qt{b}"))
            pools["kv"].append(kvp.tile([P, skv], BF16, tag="kv", name=f"kv{b}"))
            pools["vp"].append(
                vpp.tile([P, nkt_kv, H + 1], BF16, tag="vp", name=f"vp{b}")
            )
        for b in range(B_PER_CORE):
            # ones column of V' (row 64 of O'^T = softmax denominator)
            nc.gpsimd.memset(pools["vp"][b][:, :, H : H + 1], 1.0)

        # Scalar queue: weights first (small transfers, ahead of the input
        # stream in the DMA queues), then the table-load anchor exp.
        nc.scalar.dma_start(out=wkv, in_=wkv_e[:, :, :])
        nc.scalar.dma_start(out=bkv, in_=bkv_e[:, :])
        nc.scalar.dma_start(out=wq, in_=wq_e[:, :, :])
        nc.scalar.dma_start(out=bq, in_=bq_e[:, :])
        nc.scalar.activation(prime, prime, AF.Exp, bias=prime[0:1, 0:1], scale=1.0)

        # Sync queue: inputs + exp biases, in consumption order. Each load
        # is tile-granular (xt_e/xkv_e are [B, T, P, NW, 128]).
        def load_x(kind, b, t0, t1):
            dst, src = pools[kind][b], (xt_e if kind == "xt" else xkv_e)
            nc.sync.dma_start(out=dst[:, t0:t1], in_=src[b, :, t0:t1])

        t1a = min(1, nkt_kv)
        t1b = min(2, nkt_kv)
        t1c = min(4, nkt_kv)
        load_x("xkv", 0, 0, t1a)          # key tile 0: first scores
        load_x("xt", 0, 0, 4)             # Q chunk 0
        load_x("xkv", 0, t1a, t1b)        # key tile 1 (first V' pair)
        load_x("xt", 0, 4, 8)             # Q chunk 1
        for b in range(B_PER_CORE):
            nc.sync.dma_start(out=pools["ebias"][b], in_=eb_e[b])
        load_x("xkv", 0, t1b, t1c)        # key tiles 2:4
        if nkt_kv > 4:
            load_x("xkv", 0, 4, nkt_kv)   # remaining b0 keys
        load_x("xt", 0, 8, 12)            # Q chunk 2
        load_x("xt", 0, 12, 16)           # Q chunk 3
        t1d = min(5, nkt_kv)
        load_x("xkv", 1, 0, t1d)
        load_x("xt", 1, 0, 8)
        if nkt_kv > 5:
            load_x("xkv", 1, 5, nkt_kv)
        load_x("xt", 1, 8, 16)

        # --- Prologue stage lists, tile-group-driven.
        g0 = _groups(nkt_kv, [1, 2, 4, 6])
        g1 = _groups(nkt_kv, [4, 8])
        npair = (nkt_kv + 1) // 2

        def kvst(b, t0, t1):
            return functools.partial(_emit_kv_proj, nc, pools, b, t0, t1)

        def qst(b, qc):
            return functools.partial(_emit_q_proj, nc, pools, b, qc)

        def vtst(b, j):
            return functools.partial(_emit_vtrans, nc, pools, b, j, nkt_kv)

        # critical prologue of sample 0, in data-arrival order (PE wait
        # queue is 4 deep - emitting a stage long before its DMA lands
        # stalls the pipe).
        kvst(0, *g0[0])()                 # key tile 0
        qst(0, 0)()
        if len(g0) > 1:
            kvst(0, *g0[1])()             # key tile 1
        vtst(0, 0)()
        qst(0, 1)()

        # --- Greedy deadline scheduling of the remaining prologue into
        # flat iteration slots (sample 0: 0..nit-1, sample 1: nit..).
        inter = [[] for _ in range(2 * nit)]
        used = [0] * (2 * nit)

        def place(st, lo, dl):
            lo = max(1, min(lo, dl, 2 * nit - 1))
            dl = min(dl, 2 * nit - 1)
            for cap in (1, 2, 99):
                for it in range(lo, dl + 1):
                    if used[it] < cap:
                        inter[it].append(st)
                        used[it] += 1
                        return it
            raise AssertionError("no slot")

        # sample 0 remaining KV groups: group (t0, t1) must be emitted
        # before iteration t0 (scores of tile t0).
        kvslot0 = {0: 0, 1: 0}
        for c in range(2, len(g0)):
            kvslot0[c] = place(kvst(0, *g0[c]), g0[c][0] - 3, max(1, g0[c][0]))
        for qc in (2, 3):
            place(qst(0, qc), 2 * qc, nkt_kv)
        for j in range(1, npair):
            last_t = min(2 * j + 2, nkt_kv)
            src_c = next(c for c, (a, bb) in enumerate(g0) if bb >= last_t)
            place(vtst(0, j), kvslot0.get(src_c, 0) + 1, max(1, 2 * j))

        # sample 1 prologue: b1 inputs land from ~20 us (slot ~nit/2+1 of
        # the exp stream) onward.
        kvslot1 = {}
        lo_kv1 = [nit // 2 + 1, nit - 2, nit - 1]
        for c in range(len(g1)):
            dl = nit + (g1[c][0] if c else 0)
            kvslot1[c] = place(kvst(1, *g1[c]), lo_kv1[min(c, len(lo_kv1) - 1)], dl)
        for qc in range(NQC):
            lo = nit - 4 if qc < 2 else nit + 2
            place(qst(1, qc), lo, nit if qc < 2 else nit + nkt_kv)
        for j in range(npair):
            last_t = min(2 * j + 2, nkt_kv)
            src_c = next(c for c, (a, bb) in enumerate(g1) if bb >= last_t)
            place(vtst(1, j), kvslot1[src_c] + 1, nit + 2 * j)

        _emit_attention(nc, pools, 0, out_e, nkt_kv, interleave=inter[:nit])
        _emit_attention(nc, pools, 1, out_e, nkt_kv, interleave=inter[nit:])


@functools.lru_cache(maxsize=2)
def build_nc(nkt_kv: int) -> bass.Bass:
    nc = bacc.Bacc()
    xt_e = nc.declare_dram_parameter(
        "xt", [B_PER_CORE, P, NKT, NW, P], BF16, isOutput=False
    )
    xkv_e = nc.declare_dram_parameter(
        "xkv", [B_PER_CORE, P, nkt_kv, NW, P], BF16, isOutput=False
    )
    eb_e = nc.declare_dram_parameter("eb", [B_PER_CORE, P, nkt_kv], F32, isOutput=False)
    wq_e = nc.declare_dram_parameter("wq", [P, NW, H], BF16, isOutput=False)
    wkv_e = nc.declare_dram_parameter("wkv", [P, NW, P], BF16, isOutput=False)
    bq_e = nc.declare_dram_parameter("bq", [H, 1], F32, isOutput=False)
    bkv_e = nc.declare_dram_parameter("bkv", [P, 1], F32, isOutput=False)
    out_e = nc.declare_dram_parameter("out", [B_PER_CORE, H + 1, S], F32, isOutput=True)

    with tile.TileContext(nc, pool_alloc_mode="queue") as tc:
        _build(nc, tc, nkt_kv, xt_e, xkv_e, eb_e, wq_e, wkv_e, bq_e, bkv_e, out_e)
    nc.finalize()
    return nc


def _host_prep(inputs):
    """Pack the full inputs into per-core DRAM layouts (layout/dtype/
    gather prep only; all arithmetic stays on device)."""
    inp = np.asarray(inputs["input"], dtype=np.float32)      # [16, S, W]
    msk = np.asarray(inputs["mask"], dtype=np.int32)         # [16, 1, S]
    B = inp.shape[0]

    # partition-major tiled X^T: [b, p, t, wc, c] = X[b, t*128+c, wc*128+p]
    def pack_t(x):
        nt = x.shape[1] // P
        return np.ascontiguousarray(
            x.reshape(B, nt, P, NW, P).transpose(0, 4, 1, 3, 2)
        ).astype(NP_BF16)

    xt = pack_t(inp)

    # compact the keys: per sample gather the valid positions, pad to
    # whole 128-key tiles (shared across cores: SPMD)
    valid = [np.nonzero(msk[b, 0])[0] for b in range(B)]
    nv_max = max(len(v) for v in valid)
    nkt_kv = min(-(-nv_max // P), NKT)
    skv = nkt_kv * P

    xkv_rows = np.zeros((B, skv, W), dtype=np.float32)
    eb = np.full((B, skv), PAD_BIAS, dtype=np.float32)
    for b in range(B):
        v = valid[b][:skv]
        xkv_rows[b, : len(v)] = inp[b, v]
        eb[b, : len(v)] = 0.0
    xkv = pack_t(xkv_rows)
    eb = (eb + EXP_MARGIN).reshape(B, nkt_kv, P).transpose(0, 2, 1)
    eb = np.ascontiguousarray(eb)

    wq_in = np.asarray(inputs["Wq"], dtype=np.float32)
    wk = np.asarray(inputs["Wk"], dtype=np.float32)
    wv = np.asarray(inputs["Wv"], dtype=np.float32)
    wq = np.ascontiguousarray(wq_in.reshape(NW, P, H).transpose(1, 0, 2)).astype(
        NP_BF16
    )
    wkv = np.concatenate([wk, wv], axis=1).reshape(NW, P, 2 * H).transpose(1, 0, 2)
    wkv = np.ascontiguousarray(wkv).astype(NP_BF16)

    bq = np.asarray(inputs["bq"], dtype=np.float32)[:, None]
    bkv = np.concatenate(
        [np.asarray(inputs["bk"]), np.asarray(inputs["bv"])]
    ).astype(np.float32)[:, None]
    return nkt_kv, xt, xkv, eb, wq, wkv, bq, bkv


def run(inputs, trace=False, **kwargs):
    nkt_kv, xt, xkv, eb, wq, wkv, bq, bkv = _host_prep(inputs)
    nc = build_nc(nkt_kv)
    in_maps = []
    for c in range(N_CORES):
        sl = slice(B_PER_CORE * c, B_PER_CORE * (c + 1))
        in_maps.append({
            "xt": xt[sl], "xkv": xkv[sl], "eb": eb[sl],
            "wq": wq, "wkv": wkv, "bq": bq, "bkv": bkv,
        })
    res = run_bass_kernel_spmd(nc, in_maps, list(range(N_CORES)), trace=trace, **kwargs)
    outs = np.concatenate(
        [res.results[i]["out"] for i in range(N_CORES)], axis=0
    )  # [16, 65, 2048]
    o = outs[:, :H, :] / outs[:, H : H + 1, :]
    return np.ascontiguousarray(o.transpose(0, 2, 1)).astype(np.float32), res


def kernel(**inputs):
    out, _ = run(inputs, trace=False)
    return out


# revision 10
# speedup vs baseline: 1.1055x; 1.1055x over previous
"""Trainium2 Bass kernel for a single attention head (nn_AttentionHead).

Problem: B=16, S=2048, W=768, H=64.
  Q = input @ Wq + bq ; K = input @ Wk + bk ; V = input @ Wv + bv
  scores = Q K^T / sqrt(H), key-padding mask, softmax, out = attn @ V.

Sharding: data-parallel over batch across 8 cores (2 samples per core).

Design (per core). Two cost facts drive it: TensorE matmul time
depends only on moving columns (contraction depth is free), and ScalarE
exp costs ~1.07 us per [128, 1024] tile. Both scale with the number of
KEY tiles, and masked keys (about half) contribute nothing - so the
host compacts each sample's keys to the valid subset (padded to whole
128-key tiles; pad keys get a -100 exp bias so they are exactly zero,
making compaction bit-equivalent).

  1. Host packs X^T bf16 tile-major [B, T, P, NW, 128] (full input for
     the Q pass; compacted keys for the K/V pass), stationaries
     Wq / [Wk|Wv], biases, and the exp bias table (layout prep only).
  2. Q projection (bf16, moving X^T) -> Q^T [64, S]; K/V projection
     (bf16, packed stationary, moving X_kv^T) -> kv [K^T rows 0:64 |
     V^T rows 64:128] over SKV compacted keys. DVE bias-add evacuations.
  3. Scores transposed S^T[k, q] = K^T.T Q^T, plain bf16 matmuls with
     64-deep contraction (cost is moving columns, so depth 64 is free).
  4. exp on ScalarE out of PSUM, scale=1/8 (absorbs 1/sqrt(H); weights
     stay unscaled), bias = -2 margin or -102 for pad keys; the margin
     cancels in the final divide.
  5. V' = [V | ones] rebuilt natural per key tile by TensorE transposes
     of kv rows 64:128 (identity corner at base partition 64). Attention
     runs in two query halves of 1024 so exp uses wide (1024-col)
     instructions while PSUM fits exactly: 2 score slots [128, 1024] +
     O'^T accumulator [65, 1024] + the projection ring = 8 banks. Row 64
     of O'^T is the softmax denominator; each half is evacuated as soon
     as it completes, overlapping the next half.
  6. Sample 1's entire prologue is deadline-scheduled into sample 0's
     attention iterations so TensorE/DVE/DMA work overlaps the exp
     stream without ever being emitted ahead of its producers.
  7. Host epilogue: O = O'[:64] / O'[64], transpose to [B, S, H].

Scheduling facts (measured): every dma_start pays ~0.6 us on the
issuing sequencer plus ~0.6 us on the SHARED HWDGE block, so input
loads are merged into a few large DMAs. DMA descriptors below 512
contiguous bytes run at half bus speed, and descriptor count drives
queue-feed latency - hence the tile-major DRAM layout, which gives
1.5 KB runs (one per partition per tile) instead of 256 B ones.
Weight/bias tensors load first, from the scalar queue's HWDGE (the
GPSIMD SWDGE path would land them behind the whole input stream).
The ACT exp table load (1.28 us) is hoisted by Bacc to the head of the
scalar queue (the dummy exp guarantees an early anchor). TensorE runs
at half clock for its first ~3 us of busy time (p-state ramp), so the
first projections start as early as the DMA plan allows. PE dispatch
is in-order with a 4-deep wait queue: a stage is only emitted once its
input DMA is expected to have landed, or it parks and stalls the pipe.
"""

import bisect
import functools

import ml_dtypes
import numpy as np

import concourse.bass as bass
import concourse.bacc as bacc
import concourse.mybir as mybir
import concourse.tile as tile
from concourse.bass_utils import run_bass_kernel_spmd
from concourse.masks import make_identity

F32 = mybir.dt.float32
BF16 = mybir.dt.bfloat16
AF = mybir.ActivationFunctionType
ALU = mybir.AluOpType

P = 128
B_PER_CORE = 2
S = 2048
W = 768
H = 64
NW = W // P      # 6 contraction chunks for the projections
NKT = S // P     # 16 key tiles uncompacted
NQC = S // 512   # 4 query chunks of 512
N_CORES = 8
PAD_BIAS = -100.0   # exp bias for pad keys (exp -> 0 exactly in bf16)
EXP_MARGIN = -2.0   # global exp bias margin (cancels in the divide)
QSCALE = 0.125      # 1/sqrt(H), applied as the exp scale

NP_BF16 = ml_dtypes.bfloat16


def _groups(nt, edges):
    """Tile-index groups [t0, t1) from edge list, clipped to nt tiles."""
    es = sorted(set([0] + [e for e in edges if 0 < e < nt] + [nt]))
    return list(zip(es[:-1], es[1:]))


def _emit_q_proj(nc, pools, b, qc):
    wq, bq, xt, qt, pps = (
        pools["wq"], pools["bq"], pools["xt"][b], pools["qt"][b], pools["pps"],
    )
    ps = pps.tile([P, 512], F32, tag="pps", name=f"pq_{b}_{qc}")
    for wc in range(NW):
        nc.tensor.matmul(
            ps[0:H, :],
            wq[:, wc, :],
            xt[:, 4 * qc : 4 * qc + 4, wc, :],
            start=(wc == 0),
            stop=(wc == NW - 1),
        )
    nc.vector.tensor_scalar(
        qt[:, qc * 512 : (qc + 1) * 512], ps[0:H, :], bq, None, ALU.add
    )


def _emit_kv_proj(nc, pools, b, t0, t1):
    """Project key tiles [t0, t1) (at most 4: moving free <= 512)."""
    wkv, bkv, xkv, kv, pps = (
        pools["wkv"], pools["bkv"], pools["xkv"][b], pools["kv"][b], pools["pps"],
    )
    n = (t1 - t0) * P
    ps = pps.tile([P, 512], F32, tag="pps", name=f"pkv_{b}_{t0}")
    for wc in range(NW):
        nc.tensor.matmul(
            ps[:, 0:n],
            wkv[:, wc, :],
            xkv[:, t0:t1, wc, :],
            start=(wc == 0),
            stop=(wc == NW - 1),
        )
    nc.vector.tensor_scalar(kv[:, t0 * P : t1 * P], ps[:, 0:n], bkv, None, ALU.add)


def _emit_vtrans(nc, pools, b, j, nkt_kv):
    """Transpose kv rows 64:128 (V^T) for key-tile pair (2j, 2j+1) into
    natural bf16 V' tiles; the last pair may hold a single tile."""
    kv, vp, ident, pps = (
        pools["kv"][b], pools["vp"][b], pools["ident"], pools["pps"],
    )
    n = min(2, nkt_kv - 2 * j)
    pst = pps.tile([P, P], BF16, tag="pps", name=f"pvt_{b}_{j}")
    for i in range(n):
        kt = 2 * j + i
        nc.tensor.transpose(
            pst[:, i * H : (i + 1) * H],
            kv[H:P, kt * P : (kt + 1) * P],
            ident[H:P, H:P],
        )
    nc.vector.tensor_copy(
        vp[:, 2 * j : 2 * j + n, 0:H],
        pst[:, 0 : n * H].rearrange("p (i h) -> p i h", h=H),
    )


def _emit_attention(nc, pools, b, out_e, nkt_kv, interleave=()):
    """Score -> exp -> PV loops for sample b, split into two query halves
    of 1024 (PSUM: two 1024-wide score slots + one [65, 1024] output
    accumulator + the projection ring = exactly 8 banks). interleave is a
    flat list over the 2*nkt_kv iterations; interleave[it] thunks are
    emitted right after that iteration's score matmuls (the other
    sample's prologue, filling TensorE under the exp stream - placed
    after the scores so a data-parked stage cannot stall them).

    PV is software-pipelined one iteration late: PV(kt) is emitted in
    iteration kt+1. Emitted in order, PV(kt) would park on exp(kt) and
    fill the PE's 4-deep wait queue, blocking scores(kt+1) behind it and
    stretching every iteration to exp+scores instead of max(exp, PE)."""
    qt, kv, vp, ebias = (
        pools["qt"][b], pools["kv"][b], pools["vp"][b], pools["ebias"][b],
    )
    sps_p, ptp, pso_p, oup = pools["sps"], pools["ptp"], pools["pso"], pools["oup"]

    HQ = S // 2

    def emit_pv(pso, kt, pt):
        for qi in range(2):
            nc.tensor.matmul(
                pso[:, qi * 512 : (qi + 1) * 512],
                vp[:, kt, :],
                pt[:, qi * 512 : (qi + 1) * 512],
                start=(kt == 0),
                stop=(kt == nkt_kv - 1),
            )

    for half in range(2):
        pso = pso_p.tile([H + 1, HQ], F32, tag="pso", name=f"pso{b}_{half}")
        pending = None
        for kt in range(nkt_kv):
            it = half * nkt_kv + kt
            pt = ptp.tile([P, HQ], BF16, tag="pt", name=f"pt_{b}_{it}")
            sps = sps_p.tile([P, HQ], F32, tag="sps", name=f"ss_{b}_{it}")
            for qi in range(2):
                nc.tensor.matmul(
                    sps[:, qi * 512 : (qi + 1) * 512],
                    kv[0:H, kt * P : (kt + 1) * P],
                    qt[:, half * HQ + qi * 512 : half * HQ + (qi + 1) * 512],
                    start=True,
                    stop=True,
                )
            for thunk in (interleave[it] if it < len(interleave) else ()):
                thunk()
            nc.scalar.activation(
                pt, sps, AF.Exp, bias=ebias[:, kt : kt + 1], scale=QSCALE
            )
            if pending is not None:
                emit_pv(pso, *pending)
            pending = (kt, pt)
        emit_pv(pso, *pending)
        # evacuate this half right away (overlaps the next half / sample);
        # output DMA on the sync queue (HWDGE) - gpsimd SWDGE costs ~1 us
        # of Pool engine time per DMA and would stretch the tail.
        ou = oup.tile([H + 1, HQ], F32, tag="ou", name=f"ou{b}_{half}")
        for qi in range(2):
            sl = slice(qi * 512, (qi + 1) * 512)
            osl = slice(half * HQ + qi * 512, half * HQ + (qi + 1) * 512)
            nc.vector.tensor_copy(ou[:, sl], pso[:, sl])
            nc.sync.dma_start(out=out_e[b, :, osl], in_=ou[:, sl])


def _build(nc, tc, nkt_kv, xt_e, xkv_e, eb_e, wq_e, wkv_e, bq_e, bkv_e, out_e):
    skv = nkt_kv * P
    nit = 2 * nkt_kv
    with (
        tc.tile_pool(name="const", bufs=1) as cpool,
        tc.tile_pool(name="xtp", bufs=2) as xtp,
        tc.tile_pool(name="xkvp", bufs=2) as xkvp,
        tc.tile_pool(name="qtp", bufs=2) as qtp,
        tc.tile_pool(name="kvp", bufs=2) as kvp,
        tc.tile_pool(name="vpp", bufs=2) as vpp,
        tc.tile_pool(name="ptp", bufs=2) as ptp,
        tc.tile_pool(name="oup", bufs=2) as oup,
        tc.tile_pool(name="ebp", bufs=2) as ebp,
        tc.tile_pool(name="sps", bufs=2, space="PSUM") as sps_p,  # 2x[128,1024]
        tc.tile_pool(name="pps", bufs=2, space="PSUM") as pps,
        tc.tile_pool(name="psop", bufs=1, space="PSUM") as pso_p,
    ):
        prime = cpool.tile([1, 1], F32, name="prime", tag="prime")
        ident = cpool.tile([P, P], BF16, name="ident", tag="ident")
        wq = cpool.tile([P, NW, H], BF16, name="wq", tag="wq")
        wkv = cpool.tile([P, NW, P], BF16, name="wkv", tag="wkv")
        bq = cpool.tile([H, 1], F32, name="bq", tag="bq")
        bkv = cpool.tile([P, 1], F32, name="bkv", tag="bkv")

        # Pool queue: compute-only prep (no SWDGE DMAs -> cheap end drain)
        nc.gpsimd.memset(prime, 0.0)
        make_identity(nc, ident)

        pools = {
            "ident": ident, "wq": wq, "wkv": wkv, "bq": bq, "bkv": bkv,
            "sps": sps_p, "pps": pps, "pso": pso_p, "ptp": ptp, "oup": oup,
            "xt": [], "xkv": [], "qt": [], "kv": [], "vp": [], "ebias": [],
        }
        for b in range(B_PER_CORE):
            pools["ebias"].append(ebp.tile([P, nkt_kv], F32, tag="eb", name=f"eb{b}"))
            pools["xt"].append(
                xtp.tile([P, NKT, NW, P], BF16, tag="xt", name=f"xt{b}")
            )
            pools["xkv"].append(
                xkvp.tile([P, nkt_kv, NW, P], BF16, tag="xkv", name=f"xkv{b}")
            )
            pools["qt"].append(qtp.tile([H, S], BF16, tag="qt", name=f"qt{b}"))
            pools["kv"].append(kvp.tile([P, skv], BF16, tag="kv", name=f"kv{b}"))
            pools["vp"].append(
                vpp.tile([P, nkt_kv, H + 1], BF16, tag="vp", name=f"vp{b}")
            )
        for b in range(B_PER_CORE):
            # ones column of V' (row 64 of O'^T = softmax denominator)
            nc.gpsimd.memset(pools["vp"][b][:, :, H : H + 1], 1.0)

        # Scalar queue: weights first (small transfers, ahead of the input
        # stream in the DMA queues), then the table-load anchor exp.
        nc.scalar.dma_start(out=wkv, in_=wkv_e[:, :, :])
        nc.scalar.dma_start(out=bkv, in_=bkv_e[:, :])
        nc.scalar.dma_start(out=wq, in_=wq_e[:, :, :])
        nc.scalar.dma_start(out=bq, in_=bq_e[:, :])
        nc.scalar.activation(prime, prime, AF.Exp, bias=prime[0:1, 0:1], scale=1.0)

        # Sync queue: inputs + exp biases, strictly in consumption order
        # (DMA queues are FIFO: a transfer completes only after every
        # earlier-dispatched byte). Each load is tile-granular.
        def load_x(kind, b, t0, t1):
            t0, t1 = min(t0, nkt_kv), min(t1, nkt_kv)
            if t0 < t1:
                dst, src = pools[kind][b], (xt_e if kind == "xt" else xkv_e)
                nc.sync.dma_start(out=dst[:, t0:t1], in_=src[b, :, t0:t1])

        nc.sync.dma_start(out=pools["xkv"][0][:, 0:1], in_=xkv_e[0, :, 0:1])
        nc.sync.dma_start(out=pools["xt"][0][:, 0:8], in_=xt_e[0, :, 0:8])
        for b in range(B_PER_CORE):
            nc.sync.dma_start(out=pools["ebias"][b], in_=eb_e[b])
        load_x("xkv", 0, 1, 2)
        load_x("xkv", 0, 2, 4)
        load_x("xkv", 0, 4, nkt_kv)
        nc.sync.dma_start(out=pools["xt"][0][:, 8:16], in_=xt_e[0, :, 8:16])
        nc.sync.dma_start(out=pools["xt"][1][:, 0:8], in_=xt_e[1, :, 0:8])
        load_x("xkv", 1, 0, 5)
        load_x("xkv", 1, 5, nkt_kv)
        nc.sync.dma_start(out=pools["xt"][1][:, 8:16], in_=xt_e[1, :, 8:16])

        def kvst(b, t0, t1):
            t0, t1 = min(t0, nkt_kv), min(t1, nkt_kv)
            if t0 >= t1:
                return lambda: None
            return functools.partial(_emit_kv_proj, nc, pools, b, t0, t1)

        def qst(b, qc):
            return functools.partial(_emit_q_proj, nc, pools, b, qc)

        def vtst(b, j):
            return functools.partial(_emit_vtrans, nc, pools, b, j, nkt_kv)

        # critical prologue of sample 0, in data-arrival order (PE
        # dispatch is in-order with a 4-deep wait queue - a stage emitted
        # long before its DMA lands parks and stalls the pipe).
        kvst(0, 0, 1)()                   # key tile 0
        qst(0, 0)()
        qst(0, 1)()
        kvst(0, 1, 2)()                   # key tile 1
        vtst(0, 0)()

        # Remaining prologue stages, placed at explicit flat slots
        # (sample 0 iterations: 0..nit-1, sample 1: nit..2*nit-1). A
        # stage at slot s is emitted after iteration s's score matmuls,
        # so it must produce tiles for scores(s+1)+ / the PV emitted at
        # iteration s+1 (PV is pipelined one iteration late).
        inter = [[] for _ in range(2 * nit)]

        def place(st, s):
            inter[max(0, min(s, 2 * nit - 1))].append(st)

        npair = (nkt_kv + 1) // 2

        def place_vt(b, j, s):
            if 1 <= j < npair:
                place(vtst(b, j), s)

        place(kvst(0, 2, 4), 1)
        place_vt(0, 1, 2)                 # V' tiles 2,3
        place(kvst(0, 4, 8), 3)
        place_vt(0, 2, 4)
        place_vt(0, 3, 5)
        place(kvst(0, 8, nkt_kv), 5)
        place_vt(0, 4, 6)
        place(qst(0, 2), 8)
        place(qst(0, 3), 8)

        place(qst(1, 0), nit - 4)
        place(qst(1, 1), nit - 3)
        place(kvst(1, 0, 4), nit - 1)
        place(vtst(1, 0), nit)
        place_vt(1, 1, nit + 1)
        place(kvst(1, 4, 5), nit + 1)
        place(kvst(1, 5, nkt_kv), nit + 2)
        place_vt(1, 2, nit + 3)
        place_vt(1, 3, nit + 4)
        place_vt(1, 4, nit + 6)
        place(qst(1, 2), nit + 7)
        place(qst(1, 3), nit + 8)

        _emit_attention(nc, pools, 0, out_e, nkt_kv, interleave=inter[:nit])
        _emit_attention(nc, pools, 1, out_e, nkt_kv, interleave=inter[nit:])


@functools.lru_cache(maxsize=2)
def build_nc(nkt_kv: int) -> bass.Bass:
    nc = bacc.Bacc()
    xt_e = nc.declare_dram_parameter(
        "xt", [B_PER_CORE, P, NKT, NW, P], BF16, isOutput=False
    )
    xkv_e = nc.declare_dram_parameter(
        "xkv", [B_PER_CORE, P, nkt_kv, NW, P], BF16, isOutput=False
    )
    eb_e = nc.declare_dram_parameter("eb", [B_PER_CORE, P, nkt_kv], F32, isOutput=False)
    wq_e = nc.declare_dram_parameter("wq", [P, NW, H], BF16, isOutput=False)
    wkv_e = nc.declare_dram_parameter("wkv", [P, NW, P], BF16, isOutput=False)
    bq_e = nc.declare_dram_parameter("bq", [H, 1], F32, isOutput=False)
    bkv_e = nc.declare_dram_parameter("bkv", [P, 1], F32, isOutput=False)
    out_e = nc.declare_dram_parameter("out", [B_PER_CORE, H + 1, S], F32, isOutput=True)

    with tile.TileContext(nc, pool_alloc_mode="queue") as tc:
        _build(nc, tc, nkt_kv, xt_e, xkv_e, eb_e, wq_e, wkv_e, bq_e, bkv_e, out_e)
    nc.finalize()
    return nc


def _host_prep(inputs):
    """Pack the full inputs into per-core DRAM layouts (layout/dtype/
    gather prep only; all arithmetic stays on device)."""
    inp = np.asarray(inputs["input"], dtype=np.float32)      # [16, S, W]
    msk = np.asarray(inputs["mask"], dtype=np.int32)         # [16, 1, S]
    B = inp.shape[0]

    # partition-major tiled X^T: [b, p, t, wc, c] = X[b, t*128+c, wc*128+p]
    def pack_t(x):
        nt = x.shape[1] // P
        return np.ascontiguousarray(
            x.reshape(B, nt, P, NW, P).transpose(0, 4, 1, 3, 2)
        ).astype(NP_BF16)

    xt = pack_t(inp)

    # compact the keys: per sample gather the valid positions, pad to
    # whole 128-key tiles (shared across cores: SPMD)
    valid = [np.nonzero(msk[b, 0])[0] for b in range(B)]
    nv_max = max(len(v) for v in valid)
    nkt_kv = min(-(-nv_max // P), NKT)
    skv = nkt_kv * P

    xkv_rows = np.zeros((B, skv, W), dtype=np.float32)
    eb = np.full((B, skv), PAD_BIAS, dtype=np.float32)
    for b in range(B):
        v = valid[b][:skv]
        xkv_rows[b, : len(v)] = inp[b, v]
        eb[b, : len(v)] = 0.0
    xkv = pack_t(xkv_rows)
    eb = (eb + EXP_MARGIN).reshape(B, nkt_kv, P).transpose(0, 2, 1)
    eb = np.ascontiguousarray(eb)

    wq_in = np.asarray(inputs["Wq"], dtype=np.float32)
    wk = np.asarray(inputs["Wk"], dtype=np.float32)
    wv = np.asarray(inputs["Wv"], dtype=np.float32)
    wq = np.ascontiguousarray(wq_in.reshape(NW, P, H).transpose(1, 0, 2)).astype(
        NP_BF16
    )
    wkv = np.concatenate([wk, wv], axis=1).reshape(NW, P, 2 * H).transpose(1, 0, 2)
    wkv = np.ascontiguousarray(wkv).astype(NP_BF16)

    bq = np.asarray(inputs["bq"], dtype=np.float32)[:, None]
    bkv = np.concatenate(
        [np.asarray(inputs["bk"]), np.asarray(inputs["bv"])]
    ).astype(np.float32)[:, None]
    return nkt_kv, xt, xkv, eb, wq, wkv, bq, bkv


def run(inputs, trace=False, **kwargs):
    nkt_kv, xt, xkv, eb, wq, wkv, bq, bkv = _host_prep(inputs)
    nc = build_nc(nkt_kv)
    in_maps = []
    for c in range(N_CORES):
        sl = slice(B_PER_CORE * c, B_PER_CORE * (c + 1))
        in_maps.append({
            "xt": xt[sl], "xkv": xkv[sl], "eb": eb[sl],
            "wq": wq, "wkv": wkv, "bq": bq, "bkv": bkv,
        })
    res = run_bass_kernel_spmd(nc, in_maps, list(range(N_CORES)), trace=trace, **kwargs)
    outs = np.concatenate(
        [res.results[i]["out"] for i in range(N_CORES)], axis=0
    )  # [16, 65, 2048]
    o = outs[:, :H, :] / outs[:, H : H + 1, :]
    return np.ascontiguousarray(o.transpose(0, 2, 1)).astype(np.float32), res


def kernel(**inputs):
    out, _ = run(inputs, trace=False)
    return out
